# revision 31
# baseline (speedup 1.0000x reference)
"""AdaptiveTokenSampling Trainium2 kernel.

Data-parallel over batch: each of the 8 NeuronCores processes one batch
element end-to-end (per-row gumbel argmax sampling, sort-free dedup via
presence/rank matmuls, indirect-DMA gather of the selected attn rows).

Problem shapes (hardcoded): b=8, h=12, n=1024, d=64, k=256.
"""

import sys

for _p in ("/opt/trn_rl_repo", "/root/.axon_site/_ro/trn_rl_repo"):
    if _p not in sys.path:
        sys.path.append(_p)

import numpy as np

B, H, N, D, K = 8, 12, 1024, 64, 256
KP1 = K + 1                     # 257 output tokens (cls + k)
NROWS = H * N                   # 12288 flattened attn rows per core
GROWS = H * KP1                 # 3084 gathered rows per core
GPAD = 3200                     # 25 * 128
NG = GPAD // 128                # 25 gather groups
EPS = 1e-6
NEG_BIG = -1.0e30
MASK_VAL = -float(np.finfo(np.float32).max) / 2
POLY_THRESH2 = (1.0 / 32.0) ** 2


def build_nc(debug=False):
    """Build the single-core Bass graph (same program for all 8 cores)."""
    import concourse.bacc as bacc
    import concourse.bass as bass
    import concourse.mybir as mybir
    from concourse.tile import TileContext
    from concourse.masks import make_identity, make_upper_triangular

    f32 = mybir.dt.float32
    i32 = mybir.dt.int32
    i16 = mybir.dt.int16
    u8 = mybir.dt.uint8
    u32 = mybir.dt.uint32
    Alu = mybir.AluOpType
    Act = mybir.ActivationFunctionType
    X = mybir.AxisListType.X

    nc = bacc.Bacc()

    attn = nc.declare_dram_parameter("attn", [NROWS, N], f32, isOutput=False)
    value = nc.declare_dram_parameter("value", [NROWS, D], f32, isOutput=False)
    gum = nc.declare_dram_parameter("gumbel", [K, N - 1], f32, isOutput=False)
    maskp = nc.declare_dram_parameter("mask", [N], f32, isOutput=False)
    out_attn = nc.declare_dram_parameter("out_attn", [GROWS, N], f32, isOutput=True)
    out_ids = nc.declare_dram_parameter("out_ids", [KP1], i32, isOutput=True)
    out_mask = nc.declare_dram_parameter("out_mask", [KP1], u8, isOutput=True)
    idr_scratch = nc.dram_tensor("idr_scratch", [GPAD], i16)
    if debug:
        dbg_samp = nc.declare_dram_parameter("dbg_samp", [2, 128], f32, isOutput=True)
        dbg_cnt = nc.declare_dram_parameter("dbg_cnt", [128, 8], f32, isOutput=True)
        dbg_pos = nc.declare_dram_parameter("dbg_pos", [128, 8], f32, isOutput=True)
        dbg_pref = nc.declare_dram_parameter("dbg_pref", [128, 8], f32, isOutput=True)
        dbg_sv = nc.declare_dram_parameter("dbg_sv", [8], f32, isOutput=True)
        dbg_logits = nc.declare_dram_parameter("dbg_logits", [N], f32, isOutput=True)
        dbg_score = nc.declare_dram_parameter("dbg_score", [2, 128, N], f32,
                                              isOutput=True)

    with TileContext(nc) as tc:
        with (
            tc.tile_pool(name="const", bufs=1) as cpool,
            tc.tile_pool(name="vload", bufs=2) as vpool,
            tc.tile_pool(name="work", bufs=1) as wpool,
            tc.tile_pool(name="jt", bufs=2) as jpool,
            tc.tile_pool(name="gather", bufs=3) as gpool,
            tc.tile_pool(name="psum", bufs=1, space="PSUM") as ppool,
            tc.tile_pool(name="psum2", bufs=1, space="PSUM") as p2pool,
        ):
            # ---- constants ----
            identity = cpool.tile([128, 128], f32, tag="identity")
            make_identity(nc, identity[:])
            tri = cpool.tile([128, 128], f32, tag="tri")
            make_upper_triangular(nc, tri[:], val=1.0, diag=True)  # tri[k,m]=1 iff k<=m
            ones_col = cpool.tile([128, 1], f32, tag="ones_col")
            nc.vector.memset(ones_col[:], 1.0)
            eps_col = cpool.tile([128, 1], f32, tag="eps_col")
            nc.vector.memset(eps_col[:], EPS)
            iota_i = cpool.tile([128, N], i32, tag="iota_i")
            nc.gpsimd.iota(iota_i[:], [[1, N]], channel_multiplier=0)
            iota_f = cpool.tile([128, N], f32, tag="iota_f")
            nc.vector.tensor_copy(iota_f[:], iota_i[:])
            tvals_i = cpool.tile([128, 8], i32, tag="tvals_i")
            nc.gpsimd.iota(tvals_i[:], [[128, 8]], channel_multiplier=1)
            tvals_f = cpool.tile([128, 8], f32, tag="tvals_f")
            nc.vector.tensor_copy(tvals_f[:], tvals_i[:])

            # ---- value norms: vn[p, c] = ||value[row 128c+p]||, c = 8h + t//128 ----
            vview = value[:, :].rearrange("(c p) d -> p c d", p=128)  # [128,96,64]
            vnorm = wpool.tile([128, 96], f32, tag="vnorm")
            CH = 12  # columns per chunk
            for cc in range(96 // CH):
                vt = vpool.tile([128, CH, D], f32, tag="vt")
                nc.sync.dma_start(out=vt[:, :, :], in_=vview[:, cc * CH:(cc + 1) * CH, :])
                sq = vpool.tile([128, CH, D], f32, tag="sq")
                nc.scalar.activation(sq[:, :, :], vt[:, :, :], Act.Square)
                nc.vector.tensor_reduce(
                    out=vnorm[:, cc * CH:(cc + 1) * CH], in_=sq[:, :, :],
                    axis=X, op=Alu.add,
                )
            vnr = wpool.tile([128, 96], f32, tag="vnr")
            nc.scalar.activation(vnr[:], vnorm[:], Act.Sqrt)

            # ---- cls attn row: a0n[h, t] = attn[h*1024, t] ----
            a0n = wpool.tile([12, N], f32, tag="a0n")
            a0src = attn[:, :].rearrange("(h t) n -> h t n", t=N)[:, 0, :]  # [12, N]
            nc.sync.dma_start(out=a0n[:, :], in_=a0src)
            # exclude t=0 (cls->cls) from the einsum
            nc.vector.memset(a0n[:, 0:1], 0.0)
            # transpose to a0T[p, 8h + j] via PE, 128-col chunks
            a0T = wpool.tile([128, 96], f32, tag="a0T")
            a0T_v = a0T[:].rearrange("p (h j) -> p h j", h=12)
            for j8 in range(8):
                tp = ppool.tile([128, 12], f32, tag="tp_ps")
                nc.tensor.transpose(
                    out=tp[:], in_=a0n[:, j8 * 128:(j8 + 1) * 128],
                    identity=identity[:12, :12],
                )
                nc.vector.tensor_copy(a0T_v[:, :, j8], tp[:])

            # ---- cls[p, j] = sum_h a0T * vnr ----
            prod = wpool.tile([128, 96], f32, tag="prod")
            nc.vector.tensor_tensor(out=prod[:], in0=a0T[:], in1=vnr[:], op=Alu.mult)
            cls = wpool.tile([128, 8], f32, tag="cls")
            nc.vector.tensor_reduce(
                out=cls[:], in_=prod[:].rearrange("p (h j) -> p j h", h=12),
                axis=X, op=Alu.add,
            )

            # ---- total = sum cls ; logits in column layout ----
            csum = wpool.tile([128, 1], f32, tag="csum")
            nc.vector.tensor_reduce(out=csum[:], in_=cls[:], axis=X, op=Alu.add)
            tot_ps = ppool.tile([1, 1], f32, tag="tot_ps")
            nc.tensor.matmul(tot_ps[:], lhsT=csum[:], rhs=ones_col[:, 0:1],
                             start=True, stop=True)
            tot_sb = wpool.tile([1, 1], f32, tag="tot_sb")
            nc.vector.tensor_copy(tot_sb[:], tot_ps[:])
            nc.vector.tensor_scalar(out=tot_sb[:], in0=tot_sb[:], scalar1=EPS,
                                    scalar2=None, op0=Alu.add)
            nc.vector.reciprocal(tot_sb[:], tot_sb[:])
            totb = wpool.tile([128, 1], f32, tag="totb")
            nc.gpsimd.partition_broadcast(totb[:], tot_sb[:])

            logits_c = wpool.tile([128, 8], f32, tag="logits_c")
            nc.vector.tensor_scalar(out=logits_c[:], in0=cls[:], scalar1=totb[:, 0:1],
                                    scalar2=None, op0=Alu.mult)
            nc.scalar.activation(logits_c[:], logits_c[:], Act.Ln,
                                 bias=eps_col[:, 0:1], scale=1.0)
            # mask blend: logits = logits*m + (1-m)*MASK_VAL
            mcol = wpool.tile([128, 8], f32, tag="mcol")
            msrc = maskp[:].rearrange("(j p) -> p j", p=128)  # [128, 8]
            nc.sync.dma_start(out=mcol[:, :], in_=msrc)
            lm = wpool.tile([128, 8], f32, tag="lm")
            nc.vector.tensor_tensor(out=lm[:], in0=logits_c[:], in1=mcol[:], op=Alu.mult)
            im = wpool.tile([128, 8], f32, tag="im")
            nc.vector.tensor_scalar(out=im[:], in0=mcol[:], scalar1=-MASK_VAL,
                                    scalar2=MASK_VAL, op0=Alu.mult, op1=Alu.add)
            nc.vector.tensor_tensor(out=lm[:], in0=lm[:], in1=im[:], op=Alu.add)
            nc.vector.memset(lm[0:1, 0:1], NEG_BIG)  # t=0 excluded

            # ---- transpose logits to row, broadcast over partitions ----
            lrow8 = ppool.tile([8, 128], f32, tag="lrow8_ps")
            nc.tensor.transpose(out=lrow8[:], in_=lm[:], identity=identity[:])
            lrow8_sb = wpool.tile([8, 128], f32, tag="lrow8_sb")
            nc.vector.tensor_copy(lrow8_sb[:], lrow8[:])
            lrow = wpool.tile([1, N], f32, tag="lrow")
            nc.sync.dma_start(
                out=lrow[0:1, :].rearrange("x (j p) -> x j p", p=128),
                in_=lrow8_sb[:, :],
            )
            lb = wpool.tile([128, N], f32, tag="lb")
            nc.gpsimd.partition_broadcast(lb[:], lrow[0:1, :])

            # ---- gumbel scores + argmax, 2 tiles of 128 draws ----
            cnt_tiles = []
            for jt in range(2):
                gt = jpool.tile([128, N], f32, tag="gt")
                nc.vector.memset(gt[:, 0:1], 0.0)
                nc.sync.dma_start(out=gt[:, 1:N], in_=gum[jt * 128:(jt + 1) * 128, :])
                v = jpool.tile([128, N], f32, tag="v")
                nc.vector.tensor_scalar(out=v[:], in0=gt[:], scalar1=EPS,
                                        scalar2=None, op0=Alu.add)
                x = jpool.tile([128, N], f32, tag="x")
                nc.scalar.activation(x[:], v[:], Act.Copy, bias=-1.0)
                t1a = jpool.tile([128, N], f32, tag="t1a")
                nc.scalar.activation(t1a[:], v[:], Act.Ln)
                # log1p cubic for |x| < 1/32: x*(1 + x*(-1/2 + x/3))
                h1 = jpool.tile([128, N], f32, tag="h1")
                nc.vector.tensor_scalar(out=h1[:], in0=x[:], scalar1=1.0 / 3.0,
                                        scalar2=-0.5, op0=Alu.mult, op1=Alu.add)
                nc.vector.tensor_tensor(out=h1[:], in0=h1[:], in1=x[:], op=Alu.mult)
                nc.scalar.activation(h1[:], h1[:], Act.Copy, bias=1.0)
                nc.vector.tensor_tensor(out=h1[:], in0=h1[:], in1=x[:], op=Alu.mult)
                x2 = jpool.tile([128, N], f32, tag="x2")
                nc.gpsimd.tensor_tensor(out=x2[:], in0=x[:], in1=x[:], op=Alu.mult)
                selm = jpool.tile([128, N], u8, tag="selm")
                nc.gpsimd.tensor_scalar(out=selm[:], in0=x2[:], scalar1=POLY_THRESH2,
                                        scalar2=None, op0=Alu.is_lt)
                t1 = jpool.tile([128, N], f32, tag="t1")
                nc.vector.select(t1[:], selm[:], h1[:], t1a[:])
                # score = logits - Ln(-t1 + eps)
                t2 = jpool.tile([128, N], f32, tag="t2")
                nc.scalar.activation(t2[:], t1[:], Act.Ln,
                                     bias=eps_col[:, 0:1], scale=-1.0)
                score = jpool.tile([128, N], f32, tag="score")
                nc.vector.tensor_tensor(out=score[:], in0=lb[:], in1=t2[:],
                                        op=Alu.subtract)
                # argmax (no ties: top-2 gap ~2e-4 in this dataset)
                mx8 = jpool.tile([128, 8], f32, tag="mx8")
                nc.vector.max(mx8[:], score[:])
                ix8 = jpool.tile([128, 8], u32, tag="ix8")
                nc.vector.max_index(ix8[:], mx8[:], score[:])
                idxf = jpool.tile([128, 1], f32, tag="idxf")
                nc.vector.tensor_copy(idxf[:], ix8[:, 0:1])
                oh2 = jpool.tile([128, N], f32, tag="oh2")
                nc.vector.tensor_scalar(out=oh2[:], in0=iota_f[:], scalar1=idxf[:, 0:1],
                                        scalar2=None, op0=Alu.is_equal)
                if debug:
                    nc.sync.dma_start(out=dbg_samp[jt:jt + 1, :].rearrange(
                        "x p -> p x"), in_=idxf[:, 0:1])
                    nc.sync.dma_start(out=dbg_score[jt, :, :], in_=score[:, :])
                # counts per token slot, column layout: cnt[p, j] = sum_draws oh2
                cnt_jt = p2pool.tile([128, 8], f32, tag=f"cnt_ps{jt}")
                cnt_tiles.append(cnt_jt)
                for j in range(8):
                    nc.tensor.matmul(
                        cnt_jt[:, j:j + 1],
                        lhsT=oh2[:, j * 128:(j + 1) * 128],
                        rhs=ones_col[:, 0:1],
                        start=True, stop=True,
                    )

            # ---- presence -> rank -> ids ----
            cnt_a = wpool.tile([128, 8], f32, tag="cnt_a")
            nc.vector.tensor_copy(cnt_a[:], cnt_tiles[0][:])
            cnt_sb = wpool.tile([128, 8], f32, tag="cnt_sb")
            nc.vector.tensor_tensor(out=cnt_sb[:], in0=cnt_a[:],
                                    in1=cnt_tiles[1][:], op=Alu.add)
            pres = wpool.tile([128, 8], f32, tag="pres")
            nc.vector.tensor_scalar(out=pres[:], in0=cnt_sb[:], scalar1=0.5,
                                    scalar2=None, op0=Alu.is_ge)
            pref_ps = ppool.tile([128, 8], f32, tag="pref_ps")
            nc.tensor.matmul(pref_ps[:], lhsT=tri[:], rhs=pres[:], start=True, stop=True)
            pref = wpool.tile([128, 8], f32, tag="pref")
            nc.vector.tensor_copy(pref[:], pref_ps[:])
            ct = wpool.tile([1, 8], f32, tag="ct")
            nc.sync.dma_start(out=ct[0:1, :], in_=pref[127:128, :])
            # exclusive scan of ct (8 elements)
            ta = wpool.tile([1, 8], f32, tag="ta")
            nc.vector.memset(ta[0:1, 0:1], 0.0)
            nc.vector.tensor_copy(ta[0:1, 1:8], ct[0:1, 0:7])
            t1s = wpool.tile([1, 8], f32, tag="t1s")
            nc.vector.tensor_copy(t1s[0:1, 0:1], ta[0:1, 0:1])
            nc.vector.tensor_tensor(out=t1s[0:1, 1:8], in0=ta[0:1, 1:8],
                                    in1=ta[0:1, 0:7], op=Alu.add)
            tb = wpool.tile([1, 8], f32, tag="tb")
            nc.vector.tensor_copy(tb[0:1, 0:2], t1s[0:1, 0:2])
            nc.vector.tensor_tensor(out=tb[0:1, 2:8], in0=t1s[0:1, 2:8],
                                    in1=t1s[0:1, 0:6], op=Alu.add)
            tcx = wpool.tile([1, 8], f32, tag="tcx")
            nc.vector.tensor_copy(tcx[0:1, 0:4], tb[0:1, 0:4])
            nc.vector.tensor_tensor(out=tcx[0:1, 4:8], in0=tb[0:1, 4:8],
                                    in1=tb[0:1, 0:4], op=Alu.add)
            mtot = wpool.tile([1, 1], f32, tag="mtot")
            nc.vector.tensor_reduce(out=mtot[:], in_=ct[0:1, :], axis=X, op=Alu.add)
            sv = wpool.tile([1, 8], f32, tag="sv")
            nc.vector.tensor_scalar(out=sv[:], in0=tcx[0:1, :], scalar1=mtot[0:1, 0:1],
                                    scalar2=float(K), op0=Alu.subtract, op1=Alu.add)
            svb = wpool.tile([128, 8], f32, tag="svb")
            nc.gpsimd.partition_broadcast(svb[:], sv[0:1, :])
            pos = wpool.tile([128, 8], f32, tag="pos")
            nc.vector.tensor_tensor(out=pos[:], in0=pref[:], in1=svb[:], op=Alu.add)
            mt = wpool.tile([128, 8], f32, tag="mt")
            nc.vector.tensor_tensor(out=mt[:], in0=tvals_f[:], in1=pres[:], op=Alu.mult)
            if debug:
                nc.sync.dma_start(out=dbg_cnt[:, :], in_=cnt_sb[:, :])
                nc.sync.dma_start(out=dbg_pos[:, :], in_=pos[:, :])
                nc.sync.dma_start(out=dbg_pref[:, :], in_=pref[:, :])
                nc.sync.dma_start(out=dbg_sv[:], in_=sv[0:1, :])
                nc.sync.dma_start(out=dbg_logits[:], in_=lrow[0:1, :])
            ids_ps = p2pool.tile([1, KP1], f32, tag="ids_ps")
            for j in range(8):
                ohp = wpool.tile([128, KP1], f32, tag="ohp")
                nc.vector.tensor_scalar(out=ohp[:], in0=iota_f[:, :KP1],
                                        scalar1=pos[:, j:j + 1], scalar2=None,
                                        op0=Alu.is_equal)
                nc.tensor.matmul(ids_ps[:], lhsT=mt[:, j:j + 1], rhs=ohp[:],
                                 start=(j == 0), stop=(j == 7))
            ids_f = wpool.tile([1, KP1], f32, tag="ids_f")
            nc.vector.tensor_copy(ids_f[:], ids_ps[:])
            maskf = wpool.tile([1, KP1], f32, tag="maskf")
            nc.vector.tensor_scalar(out=maskf[:], in0=ids_f[:], scalar1=0.5,
                                    scalar2=None, op0=Alu.is_ge)
            nc.vector.memset(maskf[0:1, 0:1], 1.0)
            ids_i = wpool.tile([1, KP1], i32, tag="ids_i")
            nc.vector.tensor_copy(ids_i[:], ids_f[:])
            mask_u = wpool.tile([1, KP1], u8, tag="mask_u")
            nc.vector.tensor_copy(mask_u[:], maskf[:])
            nc.sync.dma_start(out=out_ids[:], in_=ids_i[0:1, :])
            nc.sync.dma_start(out=out_mask[:], in_=mask_u[0:1, :])

            # ---- gather indices: flat = h*257 + q -> row h*1024 + ids[q] ----
            idr = wpool.tile([1, GPAD], f32, tag="idr")
            nc.vector.memset(idr[:], -1.0)  # pad tail: negative = skipped
            for hh in range(H):
                nc.vector.tensor_scalar(
                    out=idr[0:1, hh * KP1:(hh + 1) * KP1], in0=ids_f[0:1, :],
                    scalar1=float(hh * N), scalar2=None, op0=Alu.add,
                )
            # wrap to dma_gather layout: idxw[p, s] = idr[16 s + p], replicated
            # across the 8 gpsimd core groups of 16 partitions. Bounce through
            # DRAM so the partition-crossing repack is a plain strided load.
            idr16 = wpool.tile([1, GPAD], i16, tag="idr16")
            nc.vector.tensor_copy(idr16[:], idr[:])
            nc.sync.dma_start(out=idr_scratch[:], in_=idr16[0:1, :])
            idxw = wpool.tile([128, GPAD // 16], i16, tag="idxw")
            wrapped_src = idr_scratch[:].rearrange("(s p) -> p s", p=16)
            for k in range(8):
                nc.sync.dma_start(out=idxw[16 * k:16 * (k + 1), :], in_=wrapped_src)

            # ---- gather + store, chunks of 512 rows (+ final 12) ----
            CHUNK = 512
            SC = CHUNK // 16       # idx columns per chunk
            for c in range(6):
                idxc = wpool.tile([128, SC], i16, tag=f"idxc{c}")
                nc.vector.tensor_copy(idxc[:], idxw[:, c * SC:(c + 1) * SC])
                gt = gpool.tile([128, CHUNK // 128, N], f32, tag="gchunk")
                nc.gpsimd.dma_gather(
                    out_ap=gt[:, :, :], in_ap=attn[:, :], idxs_ap=idxc[:, :],
                    num_idxs=CHUNK, num_idxs_reg=CHUNK, elem_size=N,
                )
                nc.sync.dma_start(
                    out=out_attn[c * CHUNK:(c + 1) * CHUNK, :].rearrange(
                        "(cc p) n -> p cc n", p=128),
                    in_=gt[:, :, :],
                )
            idxl = wpool.tile([128, 8], i16, tag="idxl")
            nc.vector.tensor_copy(idxl[:], idxw[:, 192:200])
            gtl = gpool.tile([128, 1, N], f32, tag="gtail")
            nc.gpsimd.dma_gather(
                out_ap=gtl[:, :, :], in_ap=attn[:, :], idxs_ap=idxl[:, :],
                num_idxs=128, num_idxs_reg=GROWS - 6 * CHUNK, elem_size=N,
            )
            nc.sync.dma_start(out=out_attn[6 * CHUNK:GROWS, :],
                              in_=gtl[:GROWS - 6 * CHUNK, 0, :])

    nc.compile()
    return nc


_NC_CACHE = None
TRACE = False          # set by test harness to capture an NTFF profile
LAST_RESULT = None     # BassKernelResults of the most recent kernel() call
TRACE_DIR = None


def _get_nc():
    global _NC_CACHE
    if _NC_CACHE is None:
        _NC_CACHE = build_nc()
    return _NC_CACHE


def _install_trace_hooks():
    """Register the NTFF profile hook (missing from this image's antenv)
    and keep artifacts local instead of uploading to a bucket."""
    import types
    if "antenv.axon_hooks" not in sys.modules:
        from trn_agent_boot.trn_boot import _ntff_profile_via_ctypes
        hook = _ntff_profile_via_ctypes("/opt/axon/libaxon_pjrt.so")
        mod = types.ModuleType("antenv.axon_hooks")
        mod.get_axon_ntff_profile_hook = lambda: hook
        mod.set_axon_ntff_profile_hook = lambda h: None
        sys.modules["antenv.axon_hooks"] = mod
    from concourse import bass_utils as BU
    BU.upload_artifacts = lambda tmpdir: tmpdir


def kernel(attn, value, mask, gumbel_u, output_num_tokens):
    from concourse.bass_utils import run_bass_kernel_spmd

    attn = np.ascontiguousarray(np.asarray(attn, dtype=np.float32))
    value = np.ascontiguousarray(np.asarray(value, dtype=np.float32))
    gumbel_u = np.ascontiguousarray(np.asarray(gumbel_u, dtype=np.float32))
    mask_f = np.ascontiguousarray(np.asarray(mask).astype(np.float32))
    assert int(np.asarray(output_num_tokens)) == K

    nc = _get_nc()
    in_maps = [
        {
            "attn": attn[b].reshape(NROWS, N),
            "value": value[b].reshape(NROWS, D),
            "gumbel": gumbel_u[b],
            "mask": mask_f[b],
        }
        for b in range(B)
    ]
    kw = {}
    if TRACE:
        import tempfile
        global TRACE_DIR
        TRACE_DIR = tempfile.mkdtemp(prefix="ats_trace_")
        kw = dict(trace=True, tmpdir=TRACE_DIR)
        _install_trace_hooks()
    res = run_bass_kernel_spmd(nc, in_maps, core_ids=list(range(B)), **kw)
    global LAST_RESULT
    LAST_RESULT = res
    results = res.results
    new_attn = np.stack([r["out_attn"].reshape(H, KP1, N) for r in results])
    new_mask = np.stack([r["out_mask"].astype(bool) for r in results])
    ids = np.stack([r["out_ids"].astype(np.int32) for r in results])
    return new_attn, new_mask, ids


# revision 35
# speedup vs baseline: 1.4203x; 1.4203x over previous
"""AdaptiveTokenSampling Trainium2 kernel.

Data-parallel over batch: each of the 8 NeuronCores processes one batch
element end-to-end (per-row gumbel argmax sampling, sort-free dedup via
presence/rank matmuls, dma_gather of the selected attn rows).

Problem shapes (hardcoded): b=8, h=12, n=1024, d=64, k=256.
"""

import sys

for _p in ("/opt/trn_rl_repo", "/root/.axon_site/_ro/trn_rl_repo"):
    if _p not in sys.path:
        sys.path.append(_p)

import numpy as np

B, H, N, D, K = 8, 12, 1024, 64, 256
KP1 = K + 1                     # 257 output tokens (cls + k)
NROWS = H * N                   # 12288 flattened attn rows per core
GROWS = H * KP1                 # 3084 gathered rows per core
GPAD = 3200                     # 25 * 128
EPS = 1e-6
NEG_BIG = -1.0e30
MASK_VAL = -float(np.finfo(np.float32).max) / 2
POLY_THRESH = 1.0 / 32.0


def build_nc(debug=False):
    """Build the single-core Bass graph (same program for all 8 cores)."""
    import concourse.bacc as bacc
    import concourse.bass as bass
    import concourse.mybir as mybir
    from concourse.tile import TileContext
    from concourse.masks import make_identity, make_upper_triangular

    f32 = mybir.dt.float32
    i32 = mybir.dt.int32
    i16 = mybir.dt.int16
    u8 = mybir.dt.uint8
    Alu = mybir.AluOpType
    Act = mybir.ActivationFunctionType
    X = mybir.AxisListType.X

    nc = bacc.Bacc(num_swdge_queues=4)

    attn = nc.declare_dram_parameter("attn", [NROWS, N], f32, isOutput=False)
    value = nc.declare_dram_parameter("value", [NROWS, D], f32, isOutput=False)
    gum = nc.declare_dram_parameter("gumbel", [K, N - 1], f32, isOutput=False)
    maskp = nc.declare_dram_parameter("mask", [N], f32, isOutput=False)
    out_attn = nc.declare_dram_parameter("out_attn", [GROWS, N], f32, isOutput=True)
    out_ids = nc.declare_dram_parameter("out_ids", [KP1], i32, isOutput=True)
    out_mask = nc.declare_dram_parameter("out_mask", [KP1], u8, isOutput=True)
    idr_scratch = nc.dram_tensor("idr_scratch", [GPAD], i16)
    if debug:
        dbg_cnt = nc.declare_dram_parameter("dbg_cnt", [128, 8], f32, isOutput=True)
        dbg_pos = nc.declare_dram_parameter("dbg_pos", [128, 8], f32, isOutput=True)
        dbg_pref = nc.declare_dram_parameter("dbg_pref", [128, 8], f32, isOutput=True)
        dbg_sv = nc.declare_dram_parameter("dbg_sv", [8], f32, isOutput=True)
        dbg_logits = nc.declare_dram_parameter("dbg_logits", [N], f32, isOutput=True)
        dbg_score = nc.declare_dram_parameter("dbg_score", [2, 128, N], f32,
                                              isOutput=True)

    with TileContext(nc) as tc:
        with (
            tc.tile_pool(name="const", bufs=1) as cpool,
            tc.tile_pool(name="vload", bufs=2) as vpool,
            tc.tile_pool(name="work", bufs=1) as wpool,
            tc.tile_pool(name="jt", bufs=2) as jpool,
            tc.tile_pool(name="gather", bufs=3) as gpool,
            tc.tile_pool(name="psA", bufs=1, space="PSUM") as ppool,
            tc.tile_pool(name="psB", bufs=2, space="PSUM") as spool,
            tc.tile_pool(name="psC", bufs=1, space="PSUM") as qpool,
        ):
            # ---- constants ----
            identity = cpool.tile([128, 128], f32, tag="identity")
            make_identity(nc, identity[:])
            tri = cpool.tile([128, 128], f32, tag="tri")
            make_upper_triangular(nc, tri[:], val=1.0, diag=True)  # tri[k,m]=1 iff k<=m
            ones_col = cpool.tile([128, 1], f32, tag="ones_col")
            nc.vector.memset(ones_col[:], 1.0)
            ones_row = cpool.tile([1, 128], f32, tag="ones_row")
            nc.vector.memset(ones_row[:], 1.0)
            eps_col = cpool.tile([128, 1], f32, tag="eps_col")
            nc.vector.memset(eps_col[:], EPS)
            iota_i = cpool.tile([128, N], i32, tag="iota_i")
            nc.gpsimd.iota(iota_i[:], [[1, N]], channel_multiplier=0)
            iota_f = cpool.tile([128, N], f32, tag="iota_f")
            nc.vector.tensor_copy(iota_f[:], iota_i[:])
            tvals_i = cpool.tile([128, 8], i32, tag="tvals_i")
            nc.gpsimd.iota(tvals_i[:], [[128, 8]], channel_multiplier=1)
            tvals_f = cpool.tile([128, 8], f32, tag="tvals_f")
            nc.vector.tensor_copy(tvals_f[:], tvals_i[:])
            offs_i = cpool.tile([12, 1], i32, tag="offs_i")
            nc.gpsimd.iota(offs_i[:], [[0, 1]], channel_multiplier=N)
            offs_f = cpool.tile([12, 1], f32, tag="offs_f")
            nc.vector.tensor_copy(offs_f[:], offs_i[:])

            # ---- gumbel tiles: t2 = Ln(-(log1p-accurate Ln(u+eps)) + eps) ----
            # (independent of logits; runs while value norms load/compute)
            t2_tiles = []
            for jt in range(2):
                gt = jpool.tile([128, N], f32, tag="gt")
                nc.vector.memset(gt[:, 0:1], 0.0)
                nc.sync.dma_start(out=gt[:, 1:N], in_=gum[jt * 128:(jt + 1) * 128, :])
                nc.vector.tensor_scalar(out=gt[:], in0=gt[:], scalar1=EPS,
                                        scalar2=None, op0=Alu.add)  # v = u + eps
                x = jpool.tile([128, N], f32, tag="x")
                nc.scalar.activation(x[:], gt[:], Act.Copy, bias=-1.0)
                t1a = jpool.tile([128, N], f32, tag="t1a")
                nc.scalar.activation(t1a[:], gt[:], Act.Ln)
                # cubic log1p for |x| < 1/32: x*(1 + x*(-1/2 + x/3))
                h1 = jpool.tile([128, N], f32, tag="h1")
                nc.vector.tensor_scalar(out=h1[:], in0=x[:], scalar1=1.0 / 3.0,
                                        scalar2=-0.5, op0=Alu.mult, op1=Alu.add)
                nc.vector.tensor_tensor(out=h1[:], in0=h1[:], in1=x[:], op=Alu.mult)
                nc.scalar.activation(h1[:], h1[:], Act.Copy, bias=1.0)
                nc.vector.tensor_tensor(out=h1[:], in0=h1[:], in1=x[:], op=Alu.mult)
                nc.scalar.activation(x[:], x[:], Act.Abs)
                selm = jpool.tile([128, N], u8, tag="selm")
                nc.vector.tensor_scalar(out=selm[:], in0=x[:], scalar1=POLY_THRESH,
                                        scalar2=None, op0=Alu.is_lt)
                nc.vector.copy_predicated(out=t1a[:], mask=selm[:], data=h1[:])
                nc.scalar.activation(t1a[:], t1a[:], Act.Ln,
                                     bias=eps_col[:, 0:1], scale=-1.0)  # t2
                t2_tiles.append(t1a)

            # ---- value norms: vnorm[p, c] = ||value row 128c+p||^2, c = 8h+j ----
            vview = value[:, :].rearrange("(c p) d -> p c d", p=128)  # [128,96,64]
            vnorm = wpool.tile([128, 96], f32, tag="vnorm")
            CH = 12
            for cc in range(96 // CH):
                vt = vpool.tile([128, CH, D], f32, tag="vt")
                nc.sync.dma_start(out=vt[:, :, :], in_=vview[:, cc * CH:(cc + 1) * CH, :])
                sq = vpool.tile([128, CH, D], f32, tag="sq")
                nc.scalar.activation(sq[:, :, :], vt[:, :, :], Act.Square)
                nc.vector.tensor_reduce(
                    out=vnorm[:, cc * CH:(cc + 1) * CH], in_=sq[:, :, :],
                    axis=X, op=Alu.add,
                )
            nc.scalar.activation(vnorm[:], vnorm[:], Act.Sqrt)

            # ---- cls attn row ----
            a0n = wpool.tile([12, N], f32, tag="a0n")
            a0src = attn[:, :].rearrange("(h t) n -> h t n", t=N)[:, 0, :]  # [12, N]
            nc.sync.dma_start(out=a0n[:, :], in_=a0src)
            nc.vector.memset(a0n[:, 0:1], 0.0)  # exclude t=0
            a0T = wpool.tile([128, 96], f32, tag="a0T")
            a0T_v = a0T[:].rearrange("p (h j) -> p h j", h=12)
            for j8 in range(8):
                tp = spool.tile([128, 12], f32, tag="ps_scratch")
                nc.tensor.transpose(out=tp[:], in_=a0n[:, j8 * 128:(j8 + 1) * 128],
                                    identity=identity[:12, :12])
                nc.vector.tensor_copy(a0T_v[:, :, j8], tp[:])

            # ---- cls[p, j] = sum_h a0T * vnorm ----
            nc.vector.tensor_tensor(out=a0T[:], in0=a0T[:], in1=vnorm[:], op=Alu.mult)
            cls = wpool.tile([128, 8], f32, tag="cls")
            nc.vector.tensor_reduce(
                out=cls[:], in_=a0T[:].rearrange("p (h j) -> p j h", h=12),
                axis=X, op=Alu.add,
            )

            # ---- logits in column layout ----
            csum = wpool.tile([128, 1], f32, tag="csum")
            nc.vector.tensor_reduce(out=csum[:], in_=cls[:], axis=X, op=Alu.add)
            tot_ps = qpool.tile([1, 1], f32, tag="ps_tot")
            nc.tensor.matmul(tot_ps[:], lhsT=csum[:], rhs=ones_col[:, 0:1],
                             start=True, stop=True)
            tot_sb = wpool.tile([1, 1], f32, tag="tot_sb")
            nc.vector.tensor_copy(tot_sb[:], tot_ps[:])
            nc.vector.tensor_scalar(out=tot_sb[:], in0=tot_sb[:], scalar1=EPS,
                                    scalar2=None, op0=Alu.add)
            nc.vector.reciprocal(tot_sb[:], tot_sb[:])
            totb = wpool.tile([128, 1], f32, tag="totb")
            nc.gpsimd.partition_broadcast(totb[:], tot_sb[:])

            lm = wpool.tile([128, 8], f32, tag="lm")
            nc.vector.tensor_scalar(out=lm[:], in0=cls[:], scalar1=totb[:, 0:1],
                                    scalar2=None, op0=Alu.mult)
            nc.scalar.activation(lm[:], lm[:], Act.Ln, bias=eps_col[:, 0:1], scale=1.0)
            mcol = wpool.tile([128, 8], f32, tag="mcol")
            msrc = maskp[:].rearrange("(j p) -> p j", p=128)  # [128, 8]
            nc.sync.dma_start(out=mcol[:, :], in_=msrc)
            im = wpool.tile([128, 8], f32, tag="im")
            nc.vector.tensor_scalar(out=im[:], in0=mcol[:], scalar1=-MASK_VAL,
                                    scalar2=MASK_VAL, op0=Alu.mult, op1=Alu.add)
            nc.vector.tensor_tensor(out=lm[:], in0=lm[:], in1=mcol[:], op=Alu.mult)
            nc.vector.tensor_tensor(out=lm[:], in0=lm[:], in1=im[:], op=Alu.add)
            nc.vector.memset(lm[0:1, 0:1], NEG_BIG)  # t=0 excluded

            # ---- logits -> row -> broadcast over partitions via PE ----
            lrow8 = spool.tile([8, 128], f32, tag="ps_scratch")
            nc.tensor.transpose(out=lrow8[:], in_=lm[:], identity=identity[:])
            lrow8_sb = wpool.tile([8, 128], f32, tag="lrow8_sb")
            nc.vector.tensor_copy(lrow8_sb[:], lrow8[:])
            lrow = wpool.tile([1, N], f32, tag="lrow")
            nc.sync.dma_start(
                out=lrow[0:1, :].rearrange("x (j p) -> x j p", p=128),
                in_=lrow8_sb[:, :],
            )
            lb_ps = ppool.tile([128, N], f32, tag="ps_lb")
            for half in range(2):
                sl = slice(half * 512, (half + 1) * 512)
                nc.tensor.matmul(lb_ps[:, sl], lhsT=ones_row[0:1, :],
                                 rhs=lrow[0:1, sl], start=True, stop=True)

            # ---- scores + per-slot counts (row layout) ----
            cnt_ps = ppool.tile([1, N], f32, tag="ps_cnt")
            for jt in range(2):
                score = jpool.tile([128, N], f32, tag="gt")  # reuse slot rotation
                nc.vector.tensor_tensor(out=score[:], in0=lb_ps[:],
                                        in1=t2_tiles[jt][:], op=Alu.subtract)
                maxv = jpool.tile([128, 1], f32, tag="maxv")
                nc.vector.tensor_reduce(out=maxv[:], in_=score[:], axis=X, op=Alu.max)
                oh = jpool.tile([128, N], f32, tag="h1")
                nc.vector.tensor_scalar(out=oh[:], in0=score[:],
                                        scalar1=maxv[:, 0:1], scalar2=None,
                                        op0=Alu.is_equal)
                if debug:
                    nc.sync.dma_start(out=dbg_score[jt, :, :], in_=score[:, :])
                for half in range(2):
                    sl = slice(half * 512, (half + 1) * 512)
                    nc.tensor.matmul(cnt_ps[0:1, sl], lhsT=ones_col[:, 0:1],
                                     rhs=oh[:, sl], start=(jt == 0), stop=(jt == 1))

            # ---- presence (column layout via transpose) ----
            cnt_row = wpool.tile([1, N], f32, tag="cnt_row")
            nc.vector.tensor_copy(cnt_row[:], cnt_ps[:])
            cnt8 = wpool.tile([8, 128], f32, tag="cnt8")
            nc.sync.dma_start(
                out=cnt8[:, :],
                in_=cnt_row[0:1, :].rearrange("x (j p) -> x j p", p=128),
            )
            cntT = spool.tile([128, 8], f32, tag="ps_scratch")
            nc.tensor.transpose(out=cntT[:], in_=cnt8[:, :], identity=identity[:8, :8])
            pres = wpool.tile([128, 8], f32, tag="pres")
            nc.vector.tensor_scalar(out=pres[:], in0=cntT[:], scalar1=0.5,
                                    scalar2=None, op0=Alu.is_ge)

            # ---- rank -> position ----
            pref_ps = spool.tile([128, 8], f32, tag="ps_scratch")
            nc.tensor.matmul(pref_ps[:], lhsT=tri[:], rhs=pres[:], start=True, stop=True)
            pref = wpool.tile([128, 8], f32, tag="pref")
            nc.vector.tensor_copy(pref[:], pref_ps[:])
            ct = wpool.tile([1, 8], f32, tag="ct")
            nc.sync.dma_start(out=ct[0:1, :], in_=pref[127:128, :])
            ta = wpool.tile([1, 8], f32, tag="ta")
            nc.vector.memset(ta[0:1, 0:1], 0.0)
            nc.vector.tensor_copy(ta[0:1, 1:8], ct[0:1, 0:7])
            t1s = wpool.tile([1, 8], f32, tag="t1s")
            nc.vector.tensor_copy(t1s[0:1, 0:1], ta[0:1, 0:1])
            nc.vector.tensor_tensor(out=t1s[0:1, 1:8], in0=ta[0:1, 1:8],
                                    in1=ta[0:1, 0:7], op=Alu.add)
            tb = wpool.tile([1, 8], f32, tag="tb")
            nc.vector.tensor_copy(tb[0:1, 0:2], t1s[0:1, 0:2])
            nc.vector.tensor_tensor(out=tb[0:1, 2:8], in0=t1s[0:1, 2:8],
                                    in1=t1s[0:1, 0:6], op=Alu.add)
            tcx = wpool.tile([1, 8], f32, tag="tcx")
            nc.vector.tensor_copy(tcx[0:1, 0:4], tb[0:1, 0:4])
            nc.vector.tensor_tensor(out=tcx[0:1, 4:8], in0=tb[0:1, 4:8],
                                    in1=tb[0:1, 0:4], op=Alu.add)
            mtot = wpool.tile([1, 1], f32, tag="mtot")
            nc.vector.tensor_reduce(out=mtot[:], in_=ct[0:1, :], axis=X, op=Alu.add)
            sv = wpool.tile([1, 8], f32, tag="sv")
            nc.vector.tensor_scalar(out=sv[:], in0=tcx[0:1, :], scalar1=mtot[0:1, 0:1],
                                    scalar2=float(K), op0=Alu.subtract, op1=Alu.add)
            svb = wpool.tile([128, 8], f32, tag="svb")
            nc.gpsimd.partition_broadcast(svb[:], sv[0:1, :])
            pos = wpool.tile([128, 8], f32, tag="pos")
            nc.vector.tensor_tensor(out=pos[:], in0=pref[:], in1=svb[:], op=Alu.add)
            mt = wpool.tile([128, 8], f32, tag="mt")
            nc.vector.tensor_tensor(out=mt[:], in0=tvals_f[:], in1=pres[:], op=Alu.mult)
            if debug:
                cnt_dbg = wpool.tile([128, 8], f32, tag="cnt_dbg")
                nc.vector.tensor_copy(cnt_dbg[:], cntT[:])
                nc.sync.dma_start(out=dbg_cnt[:, :], in_=cnt_dbg[:, :])
                nc.sync.dma_start(out=dbg_pos[:, :], in_=pos[:, :])
                nc.sync.dma_start(out=dbg_pref[:, :], in_=pref[:, :])
                nc.sync.dma_start(out=dbg_sv[:], in_=sv[0:1, :])
                nc.sync.dma_start(out=dbg_logits[:], in_=lrow[0:1, :])

            # ---- scatter ids: ids[pos[p,j]] = t(p,j) for present entries ----
            ids_ps = qpool.tile([1, KP1], f32, tag="ps_ids")
            for j in range(8):
                ohp = wpool.tile([128, KP1], f32, tag="ohp")
                nc.vector.tensor_scalar(out=ohp[:], in0=iota_f[:, :KP1],
                                        scalar1=pos[:, j:j + 1], scalar2=None,
                                        op0=Alu.is_equal)
                nc.tensor.matmul(ids_ps[:], lhsT=mt[:, j:j + 1], rhs=ohp[:],
                                 start=(j == 0), stop=(j == 7))
            ids_f = wpool.tile([1, KP1], f32, tag="ids_f")
            nc.vector.tensor_copy(ids_f[:], ids_ps[:])
            maskf = wpool.tile([1, KP1], f32, tag="maskf")
            nc.vector.tensor_scalar(out=maskf[:], in0=ids_f[:], scalar1=0.5,
                                    scalar2=None, op0=Alu.is_ge)
            nc.vector.memset(maskf[0:1, 0:1], 1.0)
            ids_i = wpool.tile([1, KP1], i32, tag="ids_i")
            nc.vector.tensor_copy(ids_i[:], ids_f[:])
            mask_u = wpool.tile([1, KP1], u8, tag="mask_u")
            nc.vector.tensor_copy(mask_u[:], maskf[:])
            nc.sync.dma_start(out=out_ids[:], in_=ids_i[0:1, :])
            nc.sync.dma_start(out=out_mask[:], in_=mask_u[0:1, :])

            # ---- gather index list: flat = 257h + q -> row 1024h + ids[q] ----
            ids12 = wpool.tile([12, KP1], f32, tag="ids12")
            nc.gpsimd.partition_broadcast(ids12[:], ids_f[0:1, :])
            nc.vector.tensor_scalar(out=ids12[:], in0=ids12[:],
                                    scalar1=offs_f[:, 0:1], scalar2=None, op0=Alu.add)
            idr16 = wpool.tile([12, KP1], i16, tag="idr16")
            nc.vector.tensor_copy(idr16[:], ids12[:])
            nc.sync.dma_start(out=idr_scratch[0:GROWS], in_=idr16[:, :])
            padt = wpool.tile([1, GPAD - GROWS], i16, tag="padt")
            nc.vector.memset(padt[:], -1)
            nc.sync.dma_start(out=idr_scratch[GROWS:GPAD], in_=padt[0:1, :])
            # wrapped idxs: idxw[p, s] = idr[16 s + p], replicated over 8 groups
            idxw = wpool.tile([128, GPAD // 16], i16, tag="idxw")
            wrapped_src = idr_scratch[:].rearrange("(s p) -> p s", p=16)
            nc.sync.dma_start(out=idxw[0:16, :], in_=wrapped_src)
            for k in range(1, 8):
                nc.sync.dma_start(out=idxw[16 * k:16 * (k + 1), :], in_=idxw[0:16, :])

            # ---- gather + store, chunks of 512 rows (+ final 12) ----
            CHUNK = 512
            SC = CHUNK // 16
            for c in range(6):
                idxc = wpool.tile([128, SC], i16, tag=f"idxc{c}")
                nc.vector.tensor_copy(idxc[:], idxw[:, c * SC:(c + 1) * SC])
                gt = gpool.tile([128, CHUNK // 128, N], f32, tag="gchunk")
                nc.gpsimd.dma_gather(
                    out_ap=gt[:, :, :], in_ap=attn[:, :], idxs_ap=idxc[:, :],
                    num_idxs=CHUNK, num_idxs_reg=CHUNK, elem_size=N,
                    queue_num=c % 4,
                )
                nc.sync.dma_start(
                    out=out_attn[c * CHUNK:(c + 1) * CHUNK, :].rearrange(
                        "(cc p) n -> p cc n", p=128),
                    in_=gt[:, :, :],
                )
            idxl = wpool.tile([128, 8], i16, tag="idxl")
            nc.vector.tensor_copy(idxl[:], idxw[:, 192:200])
            gtl = gpool.tile([128, 1, N], f32, tag="gtail")
            nc.gpsimd.dma_gather(
                out_ap=gtl[:, :, :], in_ap=attn[:, :], idxs_ap=idxl[:, :],
                num_idxs=128, num_idxs_reg=GROWS - 6 * CHUNK, elem_size=N,
                queue_num=2,
            )
            nc.sync.dma_start(out=out_attn[6 * CHUNK:GROWS, :],
                              in_=gtl[:GROWS - 6 * CHUNK, 0, :])

    nc.compile()
    return nc


_NC_CACHE = None
TRACE = False          # set by test harness to capture an NTFF profile
LAST_RESULT = None     # BassKernelResults of the most recent kernel() call
TRACE_DIR = None


def _get_nc():
    global _NC_CACHE
    if _NC_CACHE is None:
        _NC_CACHE = build_nc()
    return _NC_CACHE


def _install_trace_hooks():
    """Register the NTFF profile hook (missing from this image's antenv)
    and keep artifacts local instead of uploading to a bucket."""
    import types
    if "antenv.axon_hooks" not in sys.modules:
        from trn_agent_boot.trn_boot import _ntff_profile_via_ctypes
        hook = _ntff_profile_via_ctypes("/opt/axon/libaxon_pjrt.so")
        mod = types.ModuleType("antenv.axon_hooks")
        mod.get_axon_ntff_profile_hook = lambda: hook
        mod.set_axon_ntff_profile_hook = lambda h: None
        sys.modules["antenv.axon_hooks"] = mod
    from concourse import bass_utils as BU
    BU.upload_artifacts = lambda tmpdir: tmpdir


def kernel(attn, value, mask, gumbel_u, output_num_tokens):
    from concourse.bass_utils import run_bass_kernel_spmd

    attn = np.ascontiguousarray(np.asarray(attn, dtype=np.float32))
    value = np.ascontiguousarray(np.asarray(value, dtype=np.float32))
    gumbel_u = np.ascontiguousarray(np.asarray(gumbel_u, dtype=np.float32))
    mask_f = np.ascontiguousarray(np.asarray(mask).astype(np.float32))
    assert int(np.asarray(output_num_tokens)) == K

    nc = _get_nc()
    in_maps = [
        {
            "attn": attn[b].reshape(NROWS, N),
            "value": value[b].reshape(NROWS, D),
            "gumbel": gumbel_u[b],
            "mask": mask_f[b],
        }
        for b in range(B)
    ]
    kw = {}
    if TRACE:
        import tempfile
        global TRACE_DIR
        TRACE_DIR = tempfile.mkdtemp(prefix="ats_trace_")
        kw = dict(trace=True, tmpdir=TRACE_DIR)
        _install_trace_hooks()
    res = run_bass_kernel_spmd(nc, in_maps, core_ids=list(range(B)), **kw)
    global LAST_RESULT
    LAST_RESULT = res
    results = res.results
    new_attn = np.stack([r["out_attn"].reshape(H, KP1, N) for r in results])
    new_mask = np.stack([r["out_mask"].astype(bool) for r in results])
    ids = np.stack([r["out_ids"].astype(np.int32) for r in results])
    return new_attn, new_mask, ids
